# revision 27
# baseline (speedup 1.0000x reference)
"""Distributed Trainium2 Bass kernel for llama-style GQA attention block.

B=2, S=2048, D=4096, NH=32, NKV=8, HD=128.  8 NeuronCores, tensor-parallel
over heads (4 q heads + 1 kv head per core), AllToAll to row-sharded layout
before the output projection (avoids the 67MB AllReduce).

All activations live transposed ([feature, seq]) so no on-chip transposes are
needed anywhere: scores are computed as S^T = K^T(d,k)-tiles^T @ Q^T, softmax
normalization is deferred past the PV matmul, and the per-column sums are
reduced with an all-ones [128,128] matmul on the PE (which also broadcasts
the sums to all partitions, so the reciprocal can be applied directly).

The softmax reciprocal runs on the scalar engine as exp(-ln(x)) — Ln and Exp
share one activation-table set, and this avoids the very slow single-partition
DVE reciprocal.

The output projection is fused into the attention phase: after each head's
AllToAll the head's 256 wo matmuls accumulate in PSUM over that head's eight
feature tiles and are then added into SBUF f32 accumulators, filling tensor-
engine idle time while the next head's attention (ACT-bound softmax) runs.

RoPE is applied in "block" form: the host permutes wq/wk rows within each head
(even pair-components first, odd second) so the on-chip rotate-half is a
partition half-swap (stream_shuffle) instead of a stride-2 interleave.
"""

import sys
import math
from contextlib import ExitStack

import numpy as np

sys.path.insert(0, "/opt/trn_rl_repo")

import ml_dtypes

from concourse import bacc, tile
import concourse.bass as bass
import concourse.mybir as mybir
from concourse.bass_utils import run_bass_kernel_spmd

B, S, D = 2, 2048, 4096
NH, NKV, HD = 32, 8, 128
BS = B * S
NC = 8
NHL = NH // NC          # 4 local q heads
ROWS = BS // NC         # 512 output rows per core
NSC = 8                 # seq chunks of 512 (global rows)
NDT = 32                # D / 128 contraction tiles
KTB = S // HD           # 16 k-tiles per batch
QBB = 4                 # 512-wide q blocks per batch
SCALE = 1.0 / math.sqrt(HD)

F32 = mybir.dt.float32
BF16 = mybir.dt.bfloat16
bf16 = ml_dtypes.bfloat16

# half-swap of the 128 partitions: 32 groups of 4, rotate by 16 groups
SWAP_MASK = [(i + 16) % 32 for i in range(32)]

_CACHE = {}


def _pin_exp_table_set(arch: str):
    """Make Exp resolve to the natural_log_exp_and_others table set.

    Ln and Exp live in one hardware table set, but the table-load inserter
    maps each function to a fixed set (Exp -> exp_and_others), so a kernel
    alternating Exp and Ln reloads tables on every switch (~2.6us each).
    Dropping Exp from the other sets in the (cached, shared) table dict makes
    the pass pick the combined set: one load for the whole kernel.
    """
    from concourse.hw_specs import get_activation_tables

    tables = get_activation_tables(arch)
    exp = mybir.ActivationFunctionType.Exp
    for name, fns in tables.items():
        if name != "natural_log_exp_and_others":
            fns.discard(exp)


def _build(mode: str):
    """mode: 'none' (no mask work), 'causal' (skip + fine-grain diag), 'general'."""
    nc = bacc.Bacc("TRN2", target_bir_lowering=False, debug=False, num_devices=NC)
    _pin_exp_table_set(nc.m.arch)

    xT = nc.dram_tensor("xT", [NSC * NDT * 128, 512], BF16, kind="ExternalInput")
    wqs = nc.dram_tensor("wqs", [128, NDT * 512], BF16, kind="ExternalInput")
    wks = nc.dram_tensor("wks", [128, NDT * 128], BF16, kind="ExternalInput")
    wvs = nc.dram_tensor("wvs", [128, NDT * 128], BF16, kind="ExternalInput")
    coss = nc.dram_tensor("coss", [128, S], F32, kind="ExternalInput")
    sins = nc.dram_tensor("sins", [128, S], F32, kind="ExternalInput")
    if mode == "causal":
        tri = nc.dram_tensor("tri", [128, 128], BF16, kind="ExternalInput")
    elif mode == "general":
        maskT = nc.dram_tensor("maskT", [S, S], BF16, kind="ExternalInput")
    wos = nc.dram_tensor("wos", [8 * NDT * 128, 512], BF16, kind="ExternalInput")
    out = nc.dram_tensor("out", [ROWS, D], F32, kind="ExternalOutput")

    with tile.TileContext(nc) as tc:
        with (
            tc.tile_pool(name="persist", bufs=1) as persist,
            tc.tile_pool(name="dram", bufs=1, space="DRAM") as dram,
        ):
            # persistent SBUF tensors
            qt_sb = [persist.tile([128, BS], BF16, tag=f"qt{h}", name=f"qt{h}") for h in range(NHL)]
            kt_sb = persist.tile([128, BS], BF16, tag="kt")
            v_sb = persist.tile([128, BS], BF16, tag="v")
            ones_sq = persist.tile([128, 128], BF16, tag="ones_sq")
            nc.vector.memset(ones_sq[:, :], 1.0)

            a2a_in_h = [dram.tile([NC * 128, 512], BF16, name=f"a2ain{h}")
                        for h in range(NHL)]
            a2a_out_h = [dram.tile([NC * 128, 512], BF16, name=f"a2aout{h}")
                         for h in range(NHL)]

            # ---------------- Phase 1: QKV projections + RoPE ----------------
            with (
                tc.tile_pool(name="w1", bufs=1) as w1,
                tc.tile_pool(name="xin", bufs=40) as xin,
                tc.tile_pool(name="qpsum", bufs=4, space="PSUM") as qpsum,
                tc.tile_pool(name="vpsum", bufs=3, space="PSUM") as vpsum,
                tc.tile_pool(name="rope", bufs=4) as rope,
            ):
                wq_sb = w1.tile([128, NDT * 512], BF16, tag="wq")
                wk_sb = w1.tile([128, NDT * 128], BF16, tag="wk")
                wv_sb = w1.tile([128, NDT * 128], BF16, tag="wv")
                cos_sb = w1.tile([128, S], F32, tag="cos")
                sin_sb = w1.tile([128, S], F32, tag="sin")
                # wk/wv/cos/sin ride the idle GPSIMD queue so the sync queue
                # serves only first-matmul-critical wq slices and x tiles
                nc.gpsimd.dma_start(out=wk_sb[:, :], in_=wks[:, :])
                nc.gpsimd.dma_start(out=wv_sb[:, :], in_=wvs[:, :])
                nc.gpsimd.dma_start(out=cos_sb[:, :], in_=coss[:, :])
                nc.gpsimd.dma_start(out=sin_sb[:, :], in_=sins[:, :])

                for sc in range(NSC):
                    xts = []
                    for dt in range(NDT):
                        # interleave wq slices with sc=0's x tiles: the first
                        # Q group consumes wq slice j at contraction step 4j
                        if sc == 0 and dt % 4 == 0:
                            j, w = dt // 4, NDT * 512 // 8
                            nc.sync.dma_start(
                                out=wq_sb[:, j * w:(j + 1) * w],
                                in_=wqs[:, j * w:(j + 1) * w],
                            )
                        xt = xin.tile([128, 512], BF16, tag="xt", name=f"xt{dt}")
                        g = sc * NDT + dt
                        nc.sync.dma_start(
                            out=xt[:, :], in_=xT[g * 128:(g + 1) * 128, :]
                        )
                        xts.append(xt)
                    pos = (sc % 4) * 512

                    def rope_drain(src, dst):
                        rot = rope.tile([128, 512], F32, tag="rot", name="rot")
                        t1 = rope.tile([128, 512], F32, tag="t1", name="t1")
                        nc.vector.stream_shuffle(
                            out=rot[:, :], in_=src[:, :], mask=SWAP_MASK
                        )
                        nc.vector.tensor_mul(
                            out=t1[:, :], in0=src[:, :],
                            in1=cos_sb[:, pos:pos + 512],
                        )
                        nc.vector.tensor_mul(
                            out=rot[:, :], in0=rot[:, :],
                            in1=sin_sb[:, pos:pos + 512],
                        )
                        nc.vector.tensor_add(
                            out=dst[:, sc * 512:(sc + 1) * 512],
                            in0=t1[:, :], in1=rot[:, :],
                        )

                    # each accumulation group gets its own psum tile, groups
                    # run back-to-back (interleaved groups on one tile break
                    # PSUM has_written semantics)
                    for h in range(NHL):
                        pq = qpsum.tile([128, 512], F32, tag="pq", name="pq")
                        for dt in range(NDT):
                            nc.tensor.matmul(
                                out=pq[:, :],
                                lhsT=wq_sb[:, dt * 512 + h * 128:dt * 512 + (h + 1) * 128],
                                rhs=xts[dt][:, :], start=dt == 0, stop=dt == NDT - 1,
                            )
                        rope_drain(pq, qt_sb[h])
                    pk = qpsum.tile([128, 512], F32, tag="pq", name="pk")
                    for dt in range(NDT):
                        nc.tensor.matmul(
                            out=pk[:, :],
                            lhsT=wk_sb[:, dt * 128:(dt + 1) * 128],
                            rhs=xts[dt][:, :], start=dt == 0, stop=dt == NDT - 1,
                        )
                    rope_drain(pk, kt_sb)
                    for st in range(4):
                        pv = vpsum.tile([128, 128], F32, tag="pv", name="pv")
                        for dt in range(NDT):
                            nc.tensor.matmul(
                                out=pv[:, :],
                                lhsT=xts[dt][:, st * 128:(st + 1) * 128],
                                rhs=wv_sb[:, dt * 128:(dt + 1) * 128],
                                start=dt == 0, stop=dt == NDT - 1,
                            )
                        nc.vector.tensor_copy(
                            out=v_sb[:, (sc * 4 + st) * 128:(sc * 4 + st + 1) * 128],
                            in_=pv[:, :],
                        )

            # ------------- Phase 2+3: attention fused with out-projection ----
            with (
                tc.tile_pool(name="p2sb", bufs=1) as p2sb,
                tc.tile_pool(name="probs", bufs=6) as probsp,
                tc.tile_pool(name="lnrec", bufs=3) as lnrec,
                tc.tile_pool(name="aosb", bufs=8) as aosb,
                tc.tile_pool(name="msksb", bufs=4) as msksb,
                tc.tile_pool(name="attp", bufs=16) as attp,
                tc.tile_pool(name="wop", bufs=16) as wop,
                tc.tile_pool(name="ysump", bufs=1) as ysump,
            ):
                # attention-phase PSUM pools live in an ExitStack so the last
                # head's out-projection tail can reclaim their banks for a
                # deeper yp rotation (the attention pools use all 8 banks)
                _ps = ExitStack()
                spsum = _ps.enter_context(tc.tile_pool(name="spsum", bufs=2, space="PSUM"))
                smpsum = _ps.enter_context(tc.tile_pool(name="smpsum", bufs=1, space="PSUM"))
                otpsum = _ps.enter_context(tc.tile_pool(name="otpsum", bufs=2, space="PSUM"))
                ypsum = _ps.enter_context(tc.tile_pool(name="ypsum", bufs=1, space="PSUM"))
                # wo accumulators: [token-subtile, outcol-chunk] f32; allocated
                # here so they reuse SBUF released by the phase-1 pools
                ysum = [[ysump.tile([128, 512], F32, tag=f"ys{dc}_{st}",
                                    name=f"ys{dc}_{st}")
                         for st in range(4)] for dc in range(8)]
                if mode == "causal":
                    tri_sb = p2sb.tile([128, 128], BF16, tag="tri")
                    nc.sync.dma_start(out=tri_sb[:, :], in_=tri[:, :])

                att_tiles = {}

                def wo_fetch(h):
                    """DMA this head's post-A2A attention tiles into SBUF.

                    Issued on the (otherwise idle) GPSIMD SWDGE queue: these
                    transfers wait on the collective, and on the in-order sync
                    HWDGE queue that wait would block every later DMA —
                    including the next head's softmax-output stores.
                    """
                    att_tiles[h] = []
                    for i in range(NC):
                        at = attp.tile([128, 512], BF16, tag="att", name=f"att{h}_{i}")
                        nc.gpsimd.dma_start(
                            out=at[:, :], in_=a2a_out_h[h][i * 128:(i + 1) * 128, :]
                        )
                        att_tiles[h].append(at)

                def wo_dc(h, dc, ypool=None):
                    """One out-feature chunk of head h's output projection."""
                    wots = []
                    for i in range(NC):
                        wot = wop.tile([128, 512], BF16, tag="wot")
                        g = dc * NDT + (i * 4 + h)
                        nc.sync.dma_start(
                            out=wot[:, :], in_=wos[g * 128:(g + 1) * 128, :]
                        )
                        wots.append(wot)
                    for st in range(4):
                        yp = (ypool or ypsum).tile([128, 512], F32, tag="yp", name="yp")
                        for i in range(NC):
                            nc.tensor.matmul(
                                out=yp[:, :],
                                lhsT=att_tiles[h][i][:, st * 128:(st + 1) * 128],
                                rhs=wots[i][:, :],
                                start=i == 0, stop=i == NC - 1,
                            )
                        if h == 0:
                            nc.scalar.copy(out=ysum[dc][st][:, :], in_=yp[:, :])
                        else:
                            nc.vector.tensor_add(
                                out=ysum[dc][st][:, :],
                                in0=ysum[dc][st][:, :], in1=yp[:, :],
                            )
                        if h == NHL - 1:
                            nc.sync.dma_start(
                                out=out[st * 128:(st + 1) * 128,
                                        dc * 512:(dc + 1) * 512],
                                in_=ysum[dc][st][:, :],
                            )

                # attention groups biggest-first: the long qb=3 groups cover
                # the previous head's AllToAll latency before its wo matmuls
                # enter the in-order PE stream
                g_order = [(b, qb) for qb in range(QBB - 1, -1, -1) for b in range(B)]
                wo_q = []
                for h in range(NHL):
                    wo_q = [(h - 1, dc) for dc in range(8)] if h > 0 else []
                    for slot, (b, qb) in enumerate(g_order):
                        if True:
                            if mode == "causal":
                                nkt = 4 * qb + 4
                            else:
                                nkt = KTB
                            # (kt, off) pairs; diag tiles only cover q >= k
                            kts = []
                            for kt in range(nkt):
                                if mode == "causal" and kt >= 4 * qb:
                                    off = (kt - 4 * qb) * 128
                                else:
                                    off = 0
                                kts.append((kt, off))
                            qbase = b * S + qb * 512
                            ot = otpsum.tile([128, 512], F32, tag="ot", name="ot")
                            sm = smpsum.tile([128, 512], F32, tag="sm", name="sm")
                            first = True
                            for c0 in range(0, nkt, 2):
                                chunk = kts[c0:c0 + 2]
                                ps = spsum.tile([128, 1024], F32, tag="ps", name="ps")
                                pr = probsp.tile([128, 1024], BF16, tag="pr", name="pr")
                                for m, (kt, off) in enumerate(chunk):
                                    nc.tensor.matmul(
                                        out=ps[:, m * 512 + off:(m + 1) * 512],
                                        lhsT=kt_sb[:, (b * KTB + kt) * 128:(b * KTB + kt + 1) * 128],
                                        rhs=qt_sb[h][:, qbase + off:qbase + 512],
                                        start=True, stop=True,
                                    )
                                    if mode == "general":
                                        mt = msksb.tile([128, 512], BF16, tag="mt")
                                        nc.sync.dma_start(
                                            out=mt[:, :],
                                            in_=maskT[kt * 128:(kt + 1) * 128,
                                                      qb * 512:(qb + 1) * 512],
                                        )
                                        nc.vector.tensor_add(
                                            out=ps[:, m * 512:(m + 1) * 512],
                                            in0=ps[:, m * 512:(m + 1) * 512],
                                            in1=mt[:, :],
                                        )
                                # exp over the written region(s)
                                if len(chunk) == 2 and chunk[0][1] == 0 and chunk[1][1] == 0:
                                    nc.scalar.activation(
                                        pr[:, :], ps[:, :],
                                        mybir.ActivationFunctionType.Exp,
                                        bias=0.0, scale=SCALE,
                                    )
                                else:
                                    for m, (kt, off) in enumerate(chunk):
                                        nc.scalar.activation(
                                            pr[:, m * 512 + off:(m + 1) * 512],
                                            ps[:, m * 512 + off:(m + 1) * 512],
                                            mybir.ActivationFunctionType.Exp,
                                            bias=0.0, scale=SCALE,
                                        )
                                # zero the 128-wide triangle on diag tiles
                                if mode == "causal":
                                    for m, (kt, off) in enumerate(chunk):
                                        if kt >= 4 * qb:
                                            nc.vector.tensor_mul(
                                                out=pr[:, m * 512 + off:m * 512 + off + 128],
                                                in0=pr[:, m * 512 + off:m * 512 + off + 128],
                                                in1=tri_sb[:, :],
                                            )
                                # PV pair then sum pair (same-shape runs)
                                for m, (kt, off) in enumerate(chunk):
                                    nc.tensor.matmul(
                                        out=ot[:, off:],
                                        lhsT=v_sb[:, (b * KTB + kt) * 128:(b * KTB + kt + 1) * 128],
                                        rhs=pr[:, m * 512 + off:(m + 1) * 512],
                                        start=first, stop=(c0 + 2 >= nkt) and m == len(chunk) - 1,
                                    )
                                    nc.tensor.matmul(
                                        out=sm[:, off:],
                                        lhsT=ones_sq[:, :],
                                        rhs=pr[:, m * 512 + off:(m + 1) * 512],
                                        start=first, stop=(c0 + 2 >= nkt) and m == len(chunk) - 1,
                                    )
                                    first = False
                            # epilogue: rec = exp(-ln(sum)), ao = ot * rec
                            lns = lnrec.tile([128, 512], F32, tag="lns")
                            nc.scalar.activation(
                                lns[:, :], sm[:, :],
                                mybir.ActivationFunctionType.Ln,
                                bias=0.0, scale=1.0,
                            )
                            rec = lnrec.tile([128, 512], F32, tag="rec")
                            nc.scalar.activation(
                                rec[:, :], lns[:, :],
                                mybir.ActivationFunctionType.Exp,
                                bias=0.0, scale=-1.0,
                            )
                            ao = aosb.tile([128, 512], BF16, tag="ao")
                            nc.vector.tensor_mul(
                                out=ao[:, :], in0=ot[:, :], in1=rec[:, :]
                            )
                            j = b * 4 + qb
                            nc.sync.dma_start(
                                out=a2a_in_h[h][j * 128:(j + 1) * 128, :],
                                in_=ao[:, :],
                            )
                        # previous head's out-projection chunks: independent PE
                        # work to fill exp-wait stalls in the in-order stream.
                        # Start at slot 1 so two attention groups (~30us) cover
                        # the AllToAll + fetch latency; for the last head hold
                        # back three chunks to cover its own AllToAll after the
                        # loop.
                        if wo_q and slot >= 1 and not (h == NHL - 1 and slot > 5):
                            wo_dc(*wo_q.pop(0))
                    # per-head AllToAll: overlaps with the next head's compute
                    nc.gpsimd.collective_compute(
                        "AllToAll", mybir.AluOpType.bypass,
                        ins=[a2a_in_h[h].opt()], outs=[a2a_out_h[h].opt()],
                        replica_groups=[list(range(NC))],
                    )
                    wo_fetch(h)
                    # leftover chunks at the head boundary (dc7 normally;
                    # dc5-7 of head NHL-2 cover the last head's AllToAll)
                    while wo_q:
                        wo_dc(*wo_q.pop(0))
                _ps.close()
                # last head's out-projection: reclaim the attention PSUM banks
                # for a deeper yp rotation so the 32 groups pipeline
                with tc.tile_pool(name="ypsum2", bufs=4, space="PSUM") as ypsum2:
                    for dc in range(8):
                        wo_dc(NHL - 1, dc, ypsum2)
    nc.compile()
    return nc


# within each head: 4 windows of 32 partitions = [16 re-pairs | 16 im-pairs],
# so the rotate-half is stream_shuffle's per-32-window rotation by 16.
_PERM_IDX = np.array(
    [
        2 * (w * 16 + (j if j < 16 else j - 16)) + (0 if j < 16 else 1)
        for w in range(4)
        for j in range(32)
    ]
)
_PI = np.array([w * 16 + (j if j < 16 else j - 16) for w in range(4) for j in range(32)])
_SGN = np.array(
    [(-1.0 if j < 16 else 1.0) for w in range(4) for j in range(32)], np.float32
)


def _perm_block(w):
    o = w.reshape(-1, HD, D)
    return o[:, _PERM_IDX, :].reshape(-1, D)


def _stage(x, wq, wk, wv, wo, freqs_cos, freqs_sin, mask):
    """Returns (mode, shared dict, per-core dicts)."""
    causal = np.where(np.triu(np.ones((S, S), dtype=bool), k=1), -1e9, 0.0).astype(
        np.float32
    )
    if not mask.any():
        mode = "none"
    elif np.array_equal(mask, causal):
        mode = "causal"
    else:
        mode = "general"

    xT = np.ascontiguousarray(x.reshape(BS, D).T)  # [D, BS]
    x_st = (
        xT.reshape(NDT, 128, NSC, 512).transpose(2, 0, 1, 3).reshape(NSC * NDT * 128, 512)
    ).astype(bf16)
    woT = np.ascontiguousarray(wo.T)  # [hd, Dout]
    wo_st = (
        woT.reshape(NDT, 128, 8, 512).transpose(2, 0, 1, 3).reshape(8 * NDT * 128, 512)
    ).astype(bf16)
    cosT = freqs_cos.T.astype(np.float32)  # [64, S]
    sinT = freqs_sin.T.astype(np.float32)
    cos_st = np.ascontiguousarray(cosT[_PI, :])
    sin_st = np.ascontiguousarray(sinT[_PI, :] * _SGN[:, None])

    shared = {"xT": x_st, "coss": cos_st, "sins": sin_st, "wos": wo_st}
    if mode == "causal":
        # within-diag-tile triangle: keep k (row r) <= q (col c)
        shared["tri"] = np.triu(np.ones((128, 128), np.float32)).astype(bf16)
    elif mode == "general":
        shared["maskT"] = np.ascontiguousarray(mask.T * math.sqrt(HD)).astype(bf16)

    per_core = []
    for c in range(NC):
        wq_c = _perm_block(wq[c * 512:(c + 1) * 512]).T  # [D, 512]
        wk_c = _perm_block(wk[c * 128:(c + 1) * 128]).T  # [D, 128]
        wv_c = wv[c * 128:(c + 1) * 128].T               # [D, 128]
        wq_st = wq_c.reshape(NDT, 128, 512).transpose(1, 0, 2).reshape(128, NDT * 512)
        wk_st = wk_c.reshape(NDT, 128, 128).transpose(1, 0, 2).reshape(128, NDT * 128)
        wv_st = wv_c.reshape(NDT, 128, 128).transpose(1, 0, 2).reshape(128, NDT * 128)
        per_core.append(
            {
                "wqs": np.ascontiguousarray(wq_st).astype(bf16),
                "wks": np.ascontiguousarray(wk_st).astype(bf16),
                "wvs": np.ascontiguousarray(wv_st).astype(bf16),
            }
        )
    return mode, shared, per_core


def _get_nc(mode):
    if mode not in _CACHE:
        _CACHE[mode] = _build(mode)
    return _CACHE[mode]


def kernel(x, wq, wk, wv, wo, freqs_cos, freqs_sin, mask, start_pos=0, **_kw):
    x = np.asarray(x, np.float32)
    wq = np.asarray(wq, np.float32)
    wk = np.asarray(wk, np.float32)
    wv = np.asarray(wv, np.float32)
    wo = np.asarray(wo, np.float32)
    freqs_cos = np.asarray(freqs_cos, np.float32)
    freqs_sin = np.asarray(freqs_sin, np.float32)
    mask = np.asarray(mask, np.float32)

    mode, shared, per_core = _stage(x, wq, wk, wv, wo, freqs_cos, freqs_sin, mask)
    nc = _get_nc(mode)
    in_maps = [dict(shared, **per_core[c]) for c in range(NC)]
    res = run_bass_kernel_spmd(nc, in_maps, core_ids=list(range(NC)))
    outs = [np.asarray(r["out"], np.float32) for r in res.results]
    return np.concatenate(outs, axis=0).reshape(B, S, D)
